# revision 12
# baseline (speedup 1.0000x reference)
"""Trainium2 Bass kernel for nn_ApproxROT (entropic Bregman-ADMM OT solver).

Distribution: pure data-parallel over batch B=8 -> one batch element per
NeuronCore (8 cores). No collectives. Per core the solver runs 4 unrolled
iterations; the two big matmul chains c2 @ exp(state) @ c1 run on TensorE in
bf16, all log-domain state stays fp32 on DVE/ACT/GPSIMD.

Layout per core ("R layout"): logical (N=1024, D=512) tensors are stored as
SBUF tiles [128, 8, 512]: row i lives at partition i%128, block i//128.
Matmul operands exp(state) are transposed via a DRAM bounce + DMA-xbar
transpose (bf16) into "T layout" [128, 4, 1024] for use as lhsT.

Scalar solver parameters (a0..a3, rho) are baked into the compiled graph as
immediates at call time. logsumexp is computed without max-subtraction (the
exponents live around -13, far from fp32 overflow/underflow).
"""

import sys

sys.path.insert(0, "/opt/trn_rl_repo")

import numpy as np

N, D, B = 1024, 512, 8
NT = N // 128   # 8 row blocks  (R layout)
DT4 = D // 128  # 4 row blocks  (T layout)
EPS = 1e-8

_CACHE = {}


def _apply_waitpatch():
    # This walrus build rejects >1 sync wait command per instruction
    # ("Too many sync wait commands"). Hoist extra waits onto standalone
    # InstEventSemaphore instructions on the same engine, inserted right
    # before the instruction in its basic block.
    import concourse.mybir as mybir
    from concourse.tile import TileContext

    if getattr(TileContext, "_waitpatch_applied", False):
        return

    def split_excess_waits(nc):
        for _, bbw in list(nc.bb_map.items()):
            bb = bbw.bb if hasattr(bbw, "bb") else bbw
            out = []
            changed = False
            for inst in bb.instructions:
                si = getattr(inst, "sync_info", None)
                if si is not None and si.on_wait and len(si.on_wait) > 1:
                    waits = list(si.on_wait)
                    for w in waits[:-1]:
                        ev = mybir.InstEventSemaphore(
                            name=nc.get_next_instruction_name(), ins=[], outs=[]
                        )
                        ev.engine = inst.engine
                        ev.sync_info = mybir.SyncInfo(on_wait=[w], on_update=[])
                        nc.register_instruction(ev)
                        out.append(ev)
                    si.on_wait[:] = waits[-1:]
                    changed = True
                out.append(inst)
            if changed:
                bb.instructions = out

    _orig_exit = TileContext.__exit__

    def _patched_exit(self, exc_type, exc_val, exc_tb):
        r = _orig_exit(self, exc_type, exc_val, exc_tb)
        if exc_type is None:
            split_excess_waits(self.nc)
        return r

    TileContext.__exit__ = _patched_exit
    TileContext._waitpatch_applied = True


def _build(params):
    """params: tuple of 4 (a0, a1, a2, a3, rho) float tuples."""
    import concourse.bass as bass
    import concourse.mybir as mybir
    from concourse.tile import TileContext

    _apply_waitpatch()

    F32 = mybir.dt.float32
    BF16 = mybir.dt.bfloat16
    AF = mybir.ActivationFunctionType
    OP = mybir.AluOpType

    nc = bass.Bass()
    x_d = nc.declare_dram_parameter("x", [N, D], F32, isOutput=False)
    c1_d = nc.declare_dram_parameter("c1", [D, D], F32, isOutput=False)
    c2_d = nc.declare_dram_parameter("c2", [N, N], F32, isOutput=False)
    p0_d = nc.declare_dram_parameter("p0", [1, D], F32, isOutput=False)
    q0_d = nc.declare_dram_parameter("q0", [N, 1], F32, isOutput=False)
    out_d = nc.declare_dram_parameter("out", [N, D], F32, isOutput=True)
    e_scr = [nc.dram_tensor(f"e_scr{i}", [N, D], BF16) for i in range(2)]
    c2bf_d = nc.dram_tensor("c2bf", [N, N], BF16)

    def R(dram_ap):  # DRAM (rows, cols) -> [128, rows//128, cols] view
        return dram_ap.rearrange("(t p) j -> p t j", p=128)

    with TileContext(nc) as tc:
        with (
            tc.tile_pool(name="state", bufs=1) as sp,
            tc.tile_pool(name="bf", bufs=1) as bp,
            tc.tile_pool(name="tmp", bufs=2) as tp,
            tc.tile_pool(name="small", bufs=1) as mp,
            tc.tile_pool(name="ps", bufs=2, space="PSUM") as pp,
        ):
            # ---------------- constants / loads ----------------
            xt = sp.tile([128, NT, D], BF16, tag="x")
            nc.gpsimd.dma_start(out=xt[:], in_=R(x_d))  # casting DMA (SWDGE)

            c1t = bp.tile([128, DT4, D], BF16, tag="c1")
            for u in range(DT4):
                c1stage = tp.tile([128, D], F32, tag="c2stage")
                nc.sync.dma_start(
                    out=c1stage[:], in_=c1_d[128 * u : 128 * (u + 1), :]
                )
                nc.vector.tensor_copy(c1t[:, u], c1stage[:])

            c2T = bp.tile([128, NT, N], BF16, tag="c2T")
            for t in range(NT):
                c2stage = tp.tile([128, N], F32, tag="c2stage")
                nc.sync.dma_start(
                    out=c2stage[:], in_=c2_d[128 * t : 128 * (t + 1), :]
                )
                c2bf = tp.tile([128, N], BF16, tag="c2bfstage")
                nc.vector.tensor_copy(c2bf[:], c2stage[:])
                nc.sync.dma_start(
                    out=c2bf_d[128 * t : 128 * (t + 1), :], in_=c2bf[:]
                )
            for u in range(NT):
                nc.sync.dma_start(
                    out=c2T[:, u],
                    in_=c2bf_d[:, 128 * u : 128 * (u + 1)],
                    transpose=True,
                )

            epsc = mp.tile([128, 1], F32, tag="epsc")
            nc.vector.memset(epsc[:], EPS)
            ones_k = mp.tile([128, 1], BF16, tag="ones_k")
            nc.vector.memset(ones_k[:], 1.0)
            ones_kf = mp.tile([128, 1], F32, tag="ones_kf")
            nc.vector.memset(ones_kf[:], 1.0)
            ones_m = mp.tile([1, 128], F32, tag="ones_m")
            nc.vector.memset(ones_m[:], 1.0)

            p0r = mp.tile([1, D], F32, tag="p0r")
            nc.sync.dma_start(out=p0r[:], in_=p0_d[:])
            q0r = tp.tile([1, N], F32, tag="c2stage")
            nc.sync.dma_start(out=q0r[:], in_=q0_d.rearrange("n 1 -> 1 n"))
            q0c = mp.tile([128, NT], F32, tag="q0c")
            nc.sync.dma_start(out=q0c[:], in_=q0_d.rearrange("(t p) 1 -> p t", p=128))

            log_p0 = mp.tile([1, D], F32, tag="log_p0")
            nc.scalar.activation(log_p0[:], p0r[:], AF.Ln)
            log_q0 = mp.tile([128, NT], F32, tag="log_q0")
            nc.scalar.activation(log_q0[:], q0c[:], AF.Ln, bias=epsc[:])

            # ---------------- state ----------------
            log_s = sp.tile([128, NT, D], F32, tag="log_s")
            z = sp.tile([128, NT, D], F32, tag="z")
            z1 = sp.tile([128, NT, D], F32, tag="z1")
            z2eta = sp.tile([128, NT, D], F32, tag="z2eta")  # z2 then log_eta_full
            log_mu_full = sp.tile([128, NT, D], F32, tag="log_mu_full")
            A = sp.tile([128, NT, D], F32, tag="A")   # y / y2 / mu / z-delta flow

            Es = bp.tile([128, NT, D], BF16, tag="Es")
            Et = bp.tile([128, NT, D], BF16, tag="Et")

            log_mu_row = mp.tile([1, D], F32, tag="log_mu_row")
            log_eta_col = mp.tile([128, NT], F32, tag="log_eta_col")
            rs = mp.tile([128, NT], F32, tag="rs")
            lse = mp.tile([128, NT], F32, tag="lse")
            rst = mp.tile([128, NT], F32, tag="rst")
            rs_mu = mp.tile([128, NT], F32, tag="rs_mu")
            lse_mu = mp.tile([128, NT], F32, tag="lse_mu")
            em8 = mp.tile([128, NT], F32, tag="em8")
            em8r = mp.tile([128, NT], F32, tag="em8r")
            eer = mp.tile([128, NT], F32, tag="eer")
            rcolc = mp.tile([128, NT], F32, tag="rcolc")
            lse_neg = mp.tile([128, NT], F32, tag="lse_neg")
            rplse_neg = mp.tile([128, NT], F32, tag="rplse_neg")
            lse_mu_neg = mp.tile([128, NT], F32, tag="lse_mu_neg")
            rstr_neg = mp.tile([128, NT], F32, tag="rstr_neg")
            ee8 = mp.tile([128, NT], F32, tag="ee8")
            col8 = mp.tile([128, NT], F32, tag="col8")
            pq = mp.tile([128, 1], F32, tag="pq")
            l2c = mp.tile([128, 1], F32, tag="l2c")
            sp0 = mp.tile([1, 1], F32, tag="sp0")
            l1 = mp.tile([1, 1], F32, tag="l1")
            rowscr = mp.tile([1, D], F32, tag="rowscr")
            cl_row = mp.tile([1, D], F32, tag="cl_row")
            cs_row = mp.tile([1, D], F32, tag="cs_row")

            def bcast(row_ap, out_sb, scale=1.0):
                ps = pp.tile([128, row_ap.shape[-1]], F32, tag="BC", bufs=1)
                nc.tensor.matmul(ps[:], lhsT=ones_m[:], rhs=row_ap, start=True, stop=True)
                nc.scalar.activation(out_sb, ps[:], AF.Copy, scale=scale)

            # ---------------- init ----------------
            for m in range(NT):
                ps = pp.tile([128, D], F32, tag="T2")
                nc.tensor.matmul(
                    ps[:], lhsT=q0r[:, 128 * m : 128 * (m + 1)], rhs=p0r[:],
                    start=True, stop=True,
                )
                nc.scalar.activation(log_s[:, m], ps[:], AF.Ln, bias=epsc[:])
                nc.scalar.activation(Es[:, m], ps[:], AF.Identity, bias=epsc[:])

            def chain(E_bf, scr, fscale, consume, fcopy_dve):
                """tmp2' = fscale * c2 @ E @ c1; consume(m, psum_tile).
                ET[p, m, u, r] = E[128m+r, 128u+p] (xbar per-m transpose)."""
                ET = bp.tile([128, NT, DT4, 128], BF16, tag="ET")
                for m in range(NT):
                    nc.sync.dma_start(
                        out=ET[:, m], in_=E_bf[:, m], transpose=True
                    )
                Fsb = bp.tile([128, NT, D], BF16, tag="Fsb")
                for m in range(NT):
                    psF = pp.tile([128, D], F32, tag="F")
                    for u in range(DT4):
                        nc.tensor.matmul(
                            psF[:],
                            lhsT=ET[:, m, u],
                            rhs=c1t[:, u],
                            start=(u == 0),
                            stop=(u == DT4 - 1),
                        )
                    if fcopy_dve:
                        nc.vector.tensor_scalar(
                            Fsb[:, m], psF[:], fscale, None, OP.mult
                        )
                    else:
                        nc.scalar.activation(Fsb[:, m], psF[:], AF.Copy, scale=fscale)
                for m in range(NT):
                    psT = pp.tile([128, D], F32, tag="T2")
                    for kb in range(NT):
                        nc.tensor.matmul(
                            psT[:],
                            lhsT=c2T[:, kb, 128 * m : 128 * (m + 1)],
                            rhs=Fsb[:, kb],
                            start=(kb == 0),
                            stop=(kb == NT - 1),
                        )
                    consume(m, psT)

            dacc = mp.tile([128, 1], F32, tag="dacc")  # dummy accum for TTR

            # ---------------- iterations ----------------
            for k in range(4):
                al, be, ga, de, r = params[k]
                last = k == 3
                split_prev = k in (1, 2)  # log_s state is w + eta_col bias

                # ---- v = (x - z)/r + log_s  (into A) ----
                if k == 0:
                    nc.vector.tensor_scalar(A[:], xt[:], 1.0 / r, None, OP.mult)
                else:
                    nc.vector.tensor_tensor(A[:], xt[:], z[:], OP.subtract)
                    nc.vector.tensor_scalar(A[:], A[:], 1.0 / r, None, OP.mult)
                nc.vector.tensor_tensor(A[:], A[:], log_s[:], OP.add)
                if split_prev:
                    eta_bias = log_q0 if k == 1 else log_eta_col
                    for m in range(NT):
                        nc.scalar.activation(
                            A[:, m], A[:, m], AF.Identity,
                            bias=eta_bias[:, m : m + 1],
                        )

                # ---- chain A; y = v + tmp2'; rowsum exp ----
                def consumeA(m, psT):
                    nc.vector.tensor_tensor(A[:, m], A[:, m], psT[:], OP.add)
                    scr = tp.tile([128, D], BF16, tag="escr")
                    nc.scalar.activation(
                        scr[:], A[:, m], AF.Exp, accum_out=rs[:, m : m + 1]
                    )

                chain(Es, e_scr[0], al / r, consumeA, fcopy_dve=True)
                nc.scalar.activation(lse[:], rs[:], AF.Ln)
                nc.vector.tensor_scalar(lse_neg[:], lse[:], -1.0, None, OP.mult)

                # ---- A := y + log_mu ; virtual log_t = A - lse ----
                if k <= 1:
                    mu_row = log_p0 if k == 0 else log_mu_row
                    PB = tp.tile([128, D], F32, tag="PB")
                    bcast(mu_row[:], PB[:])
                    for m in range(NT):
                        nc.vector.tensor_tensor(A[:, m], A[:, m], PB[:], OP.add)
                else:
                    nc.vector.tensor_tensor(A[:], A[:], log_mu_full[:], OP.add)

                if last:
                    # output = exp(log_t) (mask all-ones); into log_s buffer
                    for m in range(NT):
                        nc.scalar.activation(
                            log_s[:, m], A[:, m], AF.Exp,
                            bias=lse_neg[:, m : m + 1],
                        )
                    nc.sync.dma_start(out=R(out_d), in_=log_s[:])
                    break

                # ---- E_t = exp(A - lse) (+ row sums of t) ----
                for m in range(NT):
                    nc.scalar.activation(
                        Et[:, m], A[:, m], AF.Exp, bias=lse_neg[:, m : m + 1],
                        accum_out=(rst[:, m : m + 1] if k <= 1 else None),
                    )

                # ---- log_s := rp*(A - lse)  (= rp*log_t, the y2 seed) ----
                cb = 1.0 / (be + r)
                rp = r / (be + r)
                nc.vector.tensor_scalar(
                    rplse_neg[:], lse[:], -rp, None, OP.mult
                )
                for m in range(NT):
                    nc.scalar.activation(
                        log_s[:, m], A[:, m], AF.Identity, scale=rp,
                        bias=rplse_neg[:, m : m + 1],
                    )

                # ---- q2 into A ----
                if k == 0:
                    nc.vector.tensor_copy(A[:], log_s[:])
                else:
                    nc.vector.tensor_scalar(A[:], z[:], cb, None, OP.mult)
                    nc.vector.tensor_tensor(A[:], A[:], log_s[:], OP.add)

                # ---- chain B; y2 = q2 + tmp2'; Ey2 = exp(y2) ----
                Ey2 = bp.tile([128, NT, D], BF16, tag="Ey2")

                def consumeB(m, psT):
                    nc.vector.tensor_tensor(A[:, m], A[:, m], psT[:], OP.add)
                    nc.scalar.activation(Ey2[:, m], A[:, m], AF.Exp)

                chain(Et, e_scr[1], al / (be + r), consumeB, fcopy_dve=False)

                # ---- col lse ----
                psCS = pp.tile([1, D], F32, tag="CS", bufs=1)
                for kb in range(NT):
                    nc.tensor.matmul(
                        psCS[:], lhsT=ones_k[:], rhs=Ey2[:, kb],
                        start=(kb == 0), stop=(kb == NT - 1),
                    )
                nc.scalar.activation(cl_row[:], psCS[:], AF.Ln)
                CLB = tp.tile([128, D], F32, tag="PB")
                bcast(cl_row[:], CLB[:], scale=-1.0)

                # ---- log_s(-w) = y2 - clse ; Es = exp(log_s [+ eta bias]) ----
                if k <= 1:
                    eta_bias = log_q0 if k == 0 else log_eta_col
                    for m in range(NT):
                        nc.vector.tensor_tensor(log_s[:, m], A[:, m], CLB[:], OP.add)
                        nc.scalar.activation(
                            Es[:, m], log_s[:, m], AF.Exp,
                            bias=eta_bias[:, m : m + 1],
                        )
                else:
                    nc.vector.tensor_tensor(A[:], A[:], z2eta[:], OP.add)
                    for m in range(NT):
                        nc.vector.tensor_tensor(log_s[:, m], A[:, m], CLB[:], OP.add)
                        nc.scalar.activation(Es[:, m], log_s[:, m], AF.Exp)
                if k == 0:
                    psCS2 = pp.tile([1, D], F32, tag="CS", bufs=1)
                    for kb in range(NT):
                        nc.tensor.matmul(
                            psCS2[:], lhsT=ones_k[:], rhs=Es[:, kb],
                            start=(kb == 0), stop=(kb == NT - 1),
                        )
                    nc.scalar.activation(cs_row[:], psCS2[:], AF.Copy)

                # ---- z += r*(Et - Es) ----
                if k == 0:
                    nc.vector.tensor_tensor(z[:], Et[:], Es[:], OP.subtract)
                    nc.vector.tensor_scalar(z[:], z[:], r, None, OP.mult)
                else:
                    nc.vector.tensor_tensor(A[:], Et[:], Es[:], OP.subtract)
                    nc.vector.tensor_scalar(A[:], A[:], r, None, OP.mult)
                    nc.vector.tensor_tensor(z[:], z[:], A[:], OP.add)

                # ---- mu block ----
                if k == 0:
                    nc.scalar.activation(rowscr[:], p0r[:], AF.Copy, accum_out=sp0[:])
                    nc.scalar.activation(l1[:], sp0[:], AF.Ln)
                    nc.vector.tensor_scalar(
                        log_mu_row[:], log_p0[:], l1[:], None, OP.subtract
                    )
                    nc.scalar.activation(rowscr[:], log_mu_row[:], AF.Exp)
                    EB = tp.tile([128, D], F32, tag="PB")
                    bcast(rowscr[:], EB[:])
                    nc.vector.tensor_scalar(rstr_neg[:], rst[:], -r, None, OP.mult)
                    for m in range(NT):
                        nc.scalar.activation(
                            z1[:, m], EB[:], AF.Identity, scale=r,
                            bias=rstr_neg[:, m : m + 1],
                        )
                elif k in (1, 2):
                    cmu = 1.0 / (r + ga)
                    if k == 1:
                        nc.vector.tensor_scalar(
                            rowscr[:], log_p0[:], ga, None, OP.mult
                        )
                        nc.vector.tensor_scalar(
                            log_mu_row[:], log_mu_row[:], r, None, OP.mult
                        )
                        nc.vector.tensor_tensor(
                            rowscr[:], rowscr[:], log_mu_row[:], OP.add
                        )
                        RB = tp.tile([128, D], F32, tag="PB")
                        bcast(rowscr[:], RB[:])
                        for m in range(NT):
                            nc.vector.tensor_tensor(
                                A[:, m], RB[:], z1[:, m], OP.subtract
                            )
                    else:
                        nc.scalar.activation(A[:], log_mu_full[:], AF.Copy, scale=r)
                        nc.vector.tensor_tensor(A[:], A[:], z1[:], OP.subtract)
                        nc.vector.tensor_scalar(
                            rowscr[:], log_p0[:], ga, None, OP.mult
                        )
                        GB = tp.tile([128, D], F32, tag="PB")
                        bcast(rowscr[:], GB[:])
                        for m in range(NT):
                            nc.vector.tensor_tensor(A[:, m], A[:, m], GB[:], OP.add)
                    Emu = bp.tile([128, NT, D], BF16, tag="Emu")
                    for m in range(NT):
                        nc.scalar.activation(
                            Emu[:, m], A[:, m], AF.Exp, scale=cmu,
                            accum_out=rs_mu[:, m : m + 1],
                        )
                    nc.scalar.activation(lse_mu[:], rs_mu[:], AF.Ln)
                    nc.vector.tensor_scalar(
                        lse_mu_neg[:], lse_mu[:], -1.0, None, OP.mult
                    )
                    for m in range(NT):
                        nc.scalar.activation(
                            log_mu_full[:, m], A[:, m], AF.Identity, scale=cmu,
                            bias=lse_mu_neg[:, m : m + 1],
                        )
                    if k == 1:
                        nc.scalar.activation(em8[:], lse_mu[:], AF.Exp, scale=-1.0)
                        nc.vector.tensor_scalar(em8r[:], em8[:], r, None, OP.mult)
                        nc.vector.tensor_scalar(
                            rstr_neg[:], rst[:], -r, None, OP.mult
                        )
                        for m in range(NT):
                            nc.scalar.activation(
                                A[:, m], Emu[:, m], AF.Identity,
                                scale=em8r[:, m : m + 1],
                                bias=rstr_neg[:, m : m + 1],
                            )
                        nc.vector.tensor_tensor(z1[:], z1[:], A[:], OP.add)

                # ---- eta block ----
                if k == 0:
                    nc.scalar.activation(col8[:], log_q0[:], AF.Exp, accum_out=pq[:])
                    ps1 = pp.tile([1, 1], F32, tag="CS", bufs=1)
                    nc.tensor.matmul(
                        ps1[:], lhsT=ones_kf[:], rhs=pq[:], start=True, stop=True
                    )
                    nc.scalar.activation(l1[:], ps1[:], AF.Ln)
                    ps2 = pp.tile([128, 1], F32, tag="BC", bufs=1)
                    nc.tensor.matmul(
                        ps2[:], lhsT=ones_m[:], rhs=l1[:], start=True, stop=True
                    )
                    nc.scalar.activation(l2c[:], ps2[:], AF.Copy)
                    nc.vector.tensor_scalar(
                        log_eta_col[:], log_q0[:], l2c[:], None, OP.subtract
                    )
                    nc.scalar.activation(ee8[:], log_eta_col[:], AF.Exp)
                    nc.vector.tensor_scalar(eer[:], ee8[:], r, None, OP.mult)
                    CSB = tp.tile([128, D], F32, tag="PB")
                    bcast(cs_row[:], CSB[:])
                    for m in range(NT):
                        nc.scalar.activation(
                            z2eta[:, m], CSB[:], AF.Identity, scale=-r,
                            bias=eer[:, m : m + 1],
                        )
                elif k == 1:
                    ceta = 1.0 / (r + de)
                    nc.vector.tensor_scalar(col8[:], log_eta_col[:], r, None, OP.mult)
                    nc.vector.tensor_scalar(ee8[:], log_q0[:], de, None, OP.mult)
                    nc.vector.tensor_tensor(col8[:], col8[:], ee8[:], OP.add)
                    nc.vector.tensor_scalar(rcolc[:], col8[:], ceta, None, OP.mult)
                    for m in range(NT):
                        nc.scalar.activation(
                            z2eta[:, m], z2eta[:, m], AF.Identity, scale=-ceta,
                            bias=rcolc[:, m : m + 1],
                        )
                    E_eta = bp.tile([128, NT, D], BF16, tag="Ey2")
                    for m in range(NT):
                        nc.scalar.activation(E_eta[:, m], z2eta[:, m], AF.Exp)
                    psCS3 = pp.tile([1, D], F32, tag="CS", bufs=1)
                    for kb in range(NT):
                        nc.tensor.matmul(
                            psCS3[:], lhsT=ones_k[:], rhs=E_eta[:, kb],
                            start=(kb == 0), stop=(kb == NT - 1),
                        )
                    nc.scalar.activation(cl_row[:], psCS3[:], AF.Ln)
                    CLB2 = tp.tile([128, D], F32, tag="PB")
                    bcast(cl_row[:], CLB2[:], scale=-1.0)
                    for m in range(NT):
                        nc.vector.tensor_tensor(
                            z2eta[:, m], z2eta[:, m], CLB2[:], OP.add
                        )
                # k == 2: eta/z1/z2 updates are dead (never read afterwards)

    return nc


def _numpy_fallback(x, c1, c2, p0, q0, a0, a1, a2, a3, rho, mask, num):
    lse_ = lambda y, ax: np.log(np.sum(np.exp(y - np.max(y, axis=ax, keepdims=True)), axis=ax, keepdims=True)) + np.max(y, axis=ax, keepdims=True)
    log_t = np.log(q0 * p0 + EPS)
    log_s = log_t.copy()
    log_mu = np.log(p0)
    log_eta = np.log(q0 + EPS)
    log_p0 = np.log(p0)
    log_q0 = np.log(q0 + EPS)
    z = np.zeros_like(log_t)
    z1 = np.zeros_like(p0)
    z2 = np.zeros_like(q0)
    for k in range(int(num)):
        n = min(k, a1.shape[0] - 1)
        tmp2 = np.matmul(np.matmul(c2, np.exp(log_s)), c1)
        y = (x + a0[n] * tmp2 - z) / rho[n] + log_s
        log_t = (log_mu - lse_(y, 2)) + y
        tmp2 = np.matmul(np.matmul(c2, np.exp(log_t)), c1)
        y = (z + a0[n] * tmp2 + rho[n] * log_t) / (a1[n] + rho[n])
        log_s = (log_eta - lse_(y, 1)) + y
        t = np.exp(log_t) * mask
        s = np.exp(log_s) * mask
        z = z + rho[n] * (t - s)
        y = (rho[n] * log_mu + a2[n] * log_p0 - z1) / (rho[n] + a2[n])
        log_mu = y - lse_(y, 2)
        y = (rho[n] * log_eta + a3[n] * log_q0 - z2) / (rho[n] + a3[n])
        log_eta = y - lse_(y, 1)
        z1 = z1 + rho[n] * (np.exp(log_mu) - np.sum(t, axis=2, keepdims=True))
        z2 = z2 + rho[n] * (np.exp(log_eta) - np.sum(s, axis=1, keepdims=True))
    return (np.exp(log_t) * mask).astype(np.float32)


def _run(nc, x, c1, c2, p0, q0, trace=False):
    from concourse.bass_utils import run_bass_kernel_spmd

    in_maps = [
        {
            "x": np.ascontiguousarray(x[b], dtype=np.float32),
            "c1": np.ascontiguousarray(c1[b], dtype=np.float32),
            "c2": np.ascontiguousarray(c2[b], dtype=np.float32),
            "p0": np.ascontiguousarray(p0[b], dtype=np.float32),
            "q0": np.ascontiguousarray(q0[b], dtype=np.float32),
        }
        for b in range(B)
    ]
    res = run_bass_kernel_spmd(nc, in_maps, core_ids=list(range(B)), trace=trace)
    out = np.stack([res.results[b]["out"] for b in range(B)]).astype(np.float32)
    return out, res


def kernel_profiled(trace=False, **inputs):
    x = np.asarray(inputs["x"], dtype=np.float32)
    c1 = np.asarray(inputs["c1"], dtype=np.float32)
    c2 = np.asarray(inputs["c2"], dtype=np.float32)
    p0 = np.asarray(inputs["p0"], dtype=np.float32)
    q0 = np.asarray(inputs["q0"], dtype=np.float32)
    a0 = np.asarray(inputs["a0"], dtype=np.float32)
    a1 = np.asarray(inputs["a1"], dtype=np.float32)
    a2 = np.asarray(inputs["a2"], dtype=np.float32)
    a3 = np.asarray(inputs["a3"], dtype=np.float32)
    rho = np.asarray(inputs["rho"], dtype=np.float32)
    mask = np.asarray(inputs["mask"], dtype=np.float32)
    num = int(np.asarray(inputs["num"]))

    if num != 4 or not np.all(mask == 1.0) or x.shape != (B, N, D):
        out = _numpy_fallback(
            x, c1, c2, p0, q0, a0, a1, a2, a3, rho, mask, num
        )
        return out, None

    params = tuple(
        (float(a0[k]), float(a1[k]), float(a2[k]), float(a3[k]), float(rho[k]))
        for k in range(4)
    )
    key = params
    if key not in _CACHE:
        _CACHE[key] = _build(params)
    nc = _CACHE[key]
    out, res = _run(nc, x, c1, c2, p0, q0, trace=trace)
    return out, res


def kernel(**inputs):
    out, _ = kernel_profiled(trace=False, **inputs)
    return out


# revision 13
# speedup vs baseline: 1.0207x; 1.0207x over previous
"""Trainium2 Bass kernel for nn_ApproxROT (entropic Bregman-ADMM OT solver).

Distribution: pure data-parallel over batch B=8 -> one batch element per
NeuronCore (8 cores). No collectives. Per core the solver runs 4 unrolled
iterations; the two big matmul chains c2 @ exp(state) @ c1 run on TensorE in
bf16, all log-domain state stays fp32 on DVE/ACT/GPSIMD.

Layout per core ("R layout"): logical (N=1024, D=512) tensors are stored as
SBUF tiles [128, 8, 512]: row i lives at partition i%128, block i//128.
Matmul operands exp(state) are transposed via a DRAM bounce + DMA-xbar
transpose (bf16) into "T layout" [128, 4, 1024] for use as lhsT.

Scalar solver parameters (a0..a3, rho) are baked into the compiled graph as
immediates at call time. logsumexp is computed without max-subtraction (the
exponents live around -13, far from fp32 overflow/underflow).
"""

import sys

sys.path.insert(0, "/opt/trn_rl_repo")

import numpy as np

N, D, B = 1024, 512, 8
NT = N // 128   # 8 row blocks  (R layout)
DT4 = D // 128  # 4 row blocks  (T layout)
EPS = 1e-8

_CACHE = {}


def _apply_waitpatch():
    # This walrus build rejects >1 sync wait command per instruction
    # ("Too many sync wait commands"). Hoist extra waits onto standalone
    # InstEventSemaphore instructions on the same engine, inserted right
    # before the instruction in its basic block.
    import concourse.mybir as mybir
    from concourse.tile import TileContext

    if getattr(TileContext, "_waitpatch_applied", False):
        return

    def split_excess_waits(nc):
        for _, bbw in list(nc.bb_map.items()):
            bb = bbw.bb if hasattr(bbw, "bb") else bbw
            out = []
            changed = False
            for inst in bb.instructions:
                si = getattr(inst, "sync_info", None)
                if si is not None and si.on_wait and len(si.on_wait) > 1:
                    waits = list(si.on_wait)
                    for w in waits[:-1]:
                        ev = mybir.InstEventSemaphore(
                            name=nc.get_next_instruction_name(), ins=[], outs=[]
                        )
                        ev.engine = inst.engine
                        ev.sync_info = mybir.SyncInfo(on_wait=[w], on_update=[])
                        nc.register_instruction(ev)
                        out.append(ev)
                    si.on_wait[:] = waits[-1:]
                    changed = True
                out.append(inst)
            if changed:
                bb.instructions = out

    _orig_exit = TileContext.__exit__

    def _patched_exit(self, exc_type, exc_val, exc_tb):
        r = _orig_exit(self, exc_type, exc_val, exc_tb)
        if exc_type is None:
            split_excess_waits(self.nc)
        return r

    TileContext.__exit__ = _patched_exit
    TileContext._waitpatch_applied = True


def _build(params):
    """params: tuple of 4 (a0, a1, a2, a3, rho) float tuples."""
    import concourse.bass as bass
    import concourse.mybir as mybir
    from concourse.tile import TileContext

    _apply_waitpatch()

    F32 = mybir.dt.float32
    BF16 = mybir.dt.bfloat16
    AF = mybir.ActivationFunctionType
    OP = mybir.AluOpType

    nc = bass.Bass()
    x_d = nc.declare_dram_parameter("x", [N, D], F32, isOutput=False)
    c1_d = nc.declare_dram_parameter("c1", [D, D], F32, isOutput=False)
    c2_d = nc.declare_dram_parameter("c2", [N, N], F32, isOutput=False)
    p0_d = nc.declare_dram_parameter("p0", [1, D], F32, isOutput=False)
    q0_d = nc.declare_dram_parameter("q0", [N, 1], F32, isOutput=False)
    out_d = nc.declare_dram_parameter("out", [N, D], F32, isOutput=True)
    e_scr = [nc.dram_tensor(f"e_scr{i}", [N, D], BF16) for i in range(2)]
    c2bf_d = nc.dram_tensor("c2bf", [N, N], BF16)

    def R(dram_ap):  # DRAM (rows, cols) -> [128, rows//128, cols] view
        return dram_ap.rearrange("(t p) j -> p t j", p=128)

    with TileContext(nc) as tc:
        with (
            tc.tile_pool(name="state", bufs=1) as sp,
            tc.tile_pool(name="bf", bufs=1) as bp,
            tc.tile_pool(name="tmp", bufs=2) as tp,
            tc.tile_pool(name="small", bufs=1) as mp,
            tc.tile_pool(name="ps", bufs=2, space="PSUM") as pp,
        ):
            # ---------------- constants / loads ----------------
            xt = sp.tile([128, NT, D], BF16, tag="x")
            nc.gpsimd.dma_start(out=xt[:], in_=R(x_d))  # casting DMA (SWDGE)

            c1t = bp.tile([128, DT4, D], BF16, tag="c1")
            for u in range(DT4):
                c1stage = tp.tile([128, D], F32, tag="c2stage")
                nc.sync.dma_start(
                    out=c1stage[:], in_=c1_d[128 * u : 128 * (u + 1), :]
                )
                nc.vector.tensor_copy(c1t[:, u], c1stage[:])

            c2T = bp.tile([128, NT, N], BF16, tag="c2T")
            for t in range(NT):
                c2stage = tp.tile([128, N], F32, tag="c2stage")
                nc.sync.dma_start(
                    out=c2stage[:], in_=c2_d[128 * t : 128 * (t + 1), :]
                )
                c2bf = tp.tile([128, N], BF16, tag="c2bfstage")
                nc.vector.tensor_copy(c2bf[:], c2stage[:])
                nc.sync.dma_start(
                    out=c2bf_d[128 * t : 128 * (t + 1), :], in_=c2bf[:]
                )
            for u in range(NT):
                nc.sync.dma_start(
                    out=c2T[:, u],
                    in_=c2bf_d[:, 128 * u : 128 * (u + 1)],
                    transpose=True,
                )

            epsc = mp.tile([128, 1], F32, tag="epsc")
            nc.vector.memset(epsc[:], EPS)
            ones_k = mp.tile([128, 1], BF16, tag="ones_k")
            nc.vector.memset(ones_k[:], 1.0)
            ones_kf = mp.tile([128, 1], F32, tag="ones_kf")
            nc.vector.memset(ones_kf[:], 1.0)
            ones_m = mp.tile([1, 128], F32, tag="ones_m")
            nc.vector.memset(ones_m[:], 1.0)

            p0r = mp.tile([1, D], F32, tag="p0r")
            nc.sync.dma_start(out=p0r[:], in_=p0_d[:])
            q0r = tp.tile([1, N], F32, tag="c2stage")
            nc.sync.dma_start(out=q0r[:], in_=q0_d.rearrange("n 1 -> 1 n"))
            q0c = mp.tile([128, NT], F32, tag="q0c")
            nc.sync.dma_start(out=q0c[:], in_=q0_d.rearrange("(t p) 1 -> p t", p=128))

            log_p0 = mp.tile([1, D], F32, tag="log_p0")
            nc.scalar.activation(log_p0[:], p0r[:], AF.Ln)
            log_q0 = mp.tile([128, NT], F32, tag="log_q0")
            nc.scalar.activation(log_q0[:], q0c[:], AF.Ln, bias=epsc[:])

            # ---------------- state ----------------
            log_s = sp.tile([128, NT, D], F32, tag="log_s")
            z = sp.tile([128, NT, D], BF16, tag="z")
            z1 = sp.tile([128, NT, D], F32, tag="z1")
            z2eta = sp.tile([128, NT, D], F32, tag="z2eta")  # z2 then log_eta_full
            log_mu_full = sp.tile([128, NT, D], F32, tag="log_mu_full")
            A = sp.tile([128, NT, D], F32, tag="A")   # y / y2 / mu / z-delta flow

            Es = bp.tile([128, NT, D], BF16, tag="Es")
            Et = bp.tile([128, NT, D], BF16, tag="Et")

            log_mu_row = mp.tile([1, D], F32, tag="log_mu_row")
            log_eta_col = mp.tile([128, NT], F32, tag="log_eta_col")
            rs = mp.tile([128, NT], F32, tag="rs")
            lse = mp.tile([128, NT], F32, tag="lse")
            rst = mp.tile([128, NT], F32, tag="rst")
            rs_mu = mp.tile([128, NT], F32, tag="rs_mu")
            lse_mu = mp.tile([128, NT], F32, tag="lse_mu")
            em8 = mp.tile([128, NT], F32, tag="em8")
            em8r = mp.tile([128, NT], F32, tag="em8r")
            eer = mp.tile([128, NT], F32, tag="eer")
            rcolc = mp.tile([128, NT], F32, tag="rcolc")
            lse_neg = mp.tile([128, NT], F32, tag="lse_neg")
            rplse_neg = mp.tile([128, NT], F32, tag="rplse_neg")
            lse_mu_neg = mp.tile([128, NT], F32, tag="lse_mu_neg")
            rstr_neg = mp.tile([128, NT], F32, tag="rstr_neg")
            ee8 = mp.tile([128, NT], F32, tag="ee8")
            col8 = mp.tile([128, NT], F32, tag="col8")
            pq = mp.tile([128, 1], F32, tag="pq")
            l2c = mp.tile([128, 1], F32, tag="l2c")
            sp0 = mp.tile([1, 1], F32, tag="sp0")
            l1 = mp.tile([1, 1], F32, tag="l1")
            rowscr = mp.tile([1, D], F32, tag="rowscr")
            cl_row = mp.tile([1, D], F32, tag="cl_row")
            cs_row = mp.tile([1, D], F32, tag="cs_row")

            def bcast(row_ap, out_sb, scale=1.0):
                ps = pp.tile([128, row_ap.shape[-1]], F32, tag="BC", bufs=1)
                nc.tensor.matmul(ps[:], lhsT=ones_m[:], rhs=row_ap, start=True, stop=True)
                nc.scalar.activation(out_sb, ps[:], AF.Copy, scale=scale)

            # ---------------- init ----------------
            for m in range(NT):
                ps = pp.tile([128, D], F32, tag="T2")
                nc.tensor.matmul(
                    ps[:], lhsT=q0r[:, 128 * m : 128 * (m + 1)], rhs=p0r[:],
                    start=True, stop=True,
                )
                nc.scalar.activation(log_s[:, m], ps[:], AF.Ln, bias=epsc[:])
                nc.scalar.activation(Es[:, m], ps[:], AF.Identity, bias=epsc[:])

            def chain(E_bf, scr, fscale, consume, fcopy_dve):
                """tmp2' = fscale * c2 @ E @ c1; consume(m, psum_tile).
                ET[p, m, u, r] = E[128m+r, 128u+p] (xbar per-m transpose)."""
                ET = bp.tile([128, NT, DT4, 128], BF16, tag="ET")
                for m in range(NT):
                    nc.sync.dma_start(
                        out=ET[:, m], in_=E_bf[:, m], transpose=True
                    )
                Fsb = bp.tile([128, NT, D], BF16, tag="Fsb")
                for m in range(NT):
                    psF = pp.tile([128, D], F32, tag="F")
                    for u in range(DT4):
                        nc.tensor.matmul(
                            psF[:],
                            lhsT=ET[:, m, u],
                            rhs=c1t[:, u],
                            start=(u == 0),
                            stop=(u == DT4 - 1),
                        )
                    if fcopy_dve:
                        nc.vector.tensor_scalar(
                            Fsb[:, m], psF[:], fscale, None, OP.mult
                        )
                    else:
                        nc.scalar.activation(Fsb[:, m], psF[:], AF.Copy, scale=fscale)
                for m in range(NT):
                    psT = pp.tile([128, D], F32, tag="T2")
                    for kb in range(NT):
                        nc.tensor.matmul(
                            psT[:],
                            lhsT=c2T[:, kb, 128 * m : 128 * (m + 1)],
                            rhs=Fsb[:, kb],
                            start=(kb == 0),
                            stop=(kb == NT - 1),
                        )
                    consume(m, psT)

            dacc = mp.tile([128, 1], F32, tag="dacc")  # dummy accum for TTR

            # ---------------- iterations ----------------
            for k in range(4):
                al, be, ga, de, r = params[k]
                last = k == 3
                split_prev = k in (1, 2)  # log_s state is w + eta_col bias

                # ---- v = (x - z)/r + log_s  (into A) ----
                if k == 0:
                    nc.vector.tensor_scalar(A[:], xt[:], 1.0 / r, None, OP.mult)
                else:
                    dz = bp.tile([128, NT, D], BF16, tag="Ey2")
                    nc.vector.tensor_tensor(dz[:], xt[:], z[:], OP.subtract)
                    nc.scalar.activation(A[:], dz[:], AF.Copy, scale=1.0 / r)
                nc.vector.tensor_tensor(A[:], A[:], log_s[:], OP.add)
                if split_prev:
                    eta_bias = log_q0 if k == 1 else log_eta_col
                    for m in range(NT):
                        nc.scalar.activation(
                            A[:, m], A[:, m], AF.Identity,
                            bias=eta_bias[:, m : m + 1],
                        )

                # ---- chain A; y = v + tmp2'; rowsum exp ----
                def consumeA(m, psT):
                    nc.vector.tensor_tensor(A[:, m], A[:, m], psT[:], OP.add)
                    scr = tp.tile([128, D], BF16, tag="escr")
                    nc.scalar.activation(
                        scr[:], A[:, m], AF.Exp, accum_out=rs[:, m : m + 1]
                    )

                chain(Es, e_scr[0], al / r, consumeA, fcopy_dve=False)
                nc.scalar.activation(lse[:], rs[:], AF.Ln)
                nc.vector.tensor_scalar(lse_neg[:], lse[:], -1.0, None, OP.mult)

                # ---- A := y + log_mu ; virtual log_t = A - lse ----
                if k <= 1:
                    mu_row = log_p0 if k == 0 else log_mu_row
                    PB = tp.tile([128, D], F32, tag="PB")
                    bcast(mu_row[:], PB[:])
                    for m in range(NT):
                        nc.vector.tensor_tensor(A[:, m], A[:, m], PB[:], OP.add)
                else:
                    nc.vector.tensor_tensor(A[:], A[:], log_mu_full[:], OP.add)

                if last:
                    # output = exp(log_t) (mask all-ones); into log_s buffer
                    for m in range(NT):
                        nc.scalar.activation(
                            log_s[:, m], A[:, m], AF.Exp,
                            bias=lse_neg[:, m : m + 1],
                        )
                    nc.sync.dma_start(out=R(out_d), in_=log_s[:])
                    break

                # ---- E_t = exp(A - lse) (+ row sums of t) ----
                for m in range(NT):
                    nc.scalar.activation(
                        Et[:, m], A[:, m], AF.Exp, bias=lse_neg[:, m : m + 1],
                        accum_out=(rst[:, m : m + 1] if k <= 1 else None),
                    )

                # ---- log_s := rp*(A - lse)  (= rp*log_t, the y2 seed) ----
                cb = 1.0 / (be + r)
                rp = r / (be + r)
                nc.vector.tensor_scalar(
                    rplse_neg[:], lse[:], -rp, None, OP.mult
                )
                for m in range(NT):
                    nc.scalar.activation(
                        log_s[:, m], A[:, m], AF.Identity, scale=rp,
                        bias=rplse_neg[:, m : m + 1],
                    )

                # ---- q2 into A ----
                if k == 0:
                    nc.vector.tensor_copy(A[:], log_s[:])
                else:
                    nc.scalar.activation(A[:], z[:], AF.Copy, scale=cb)
                    nc.vector.tensor_tensor(A[:], A[:], log_s[:], OP.add)

                # ---- chain B; y2 = q2 + tmp2'; Ey2 = exp(y2) ----
                Ey2 = bp.tile([128, NT, D], BF16, tag="Ey2")

                def consumeB(m, psT):
                    nc.vector.tensor_tensor(A[:, m], A[:, m], psT[:], OP.add)
                    nc.scalar.activation(Ey2[:, m], A[:, m], AF.Exp)

                chain(Et, e_scr[1], al / (be + r), consumeB, fcopy_dve=False)

                # ---- col lse ----
                psCS = pp.tile([1, D], F32, tag="CS", bufs=1)
                for kb in range(NT):
                    nc.tensor.matmul(
                        psCS[:], lhsT=ones_k[:], rhs=Ey2[:, kb],
                        start=(kb == 0), stop=(kb == NT - 1),
                    )
                nc.scalar.activation(cl_row[:], psCS[:], AF.Ln)
                CLB = tp.tile([128, D], F32, tag="PB")
                bcast(cl_row[:], CLB[:], scale=-1.0)

                # ---- log_s(-w) = y2 - clse ; Es = exp(log_s [+ eta bias]) ----
                if k <= 1:
                    eta_bias = log_q0 if k == 0 else log_eta_col
                    for m in range(NT):
                        nc.vector.tensor_tensor(log_s[:, m], A[:, m], CLB[:], OP.add)
                        nc.scalar.activation(
                            Es[:, m], log_s[:, m], AF.Exp,
                            bias=eta_bias[:, m : m + 1],
                        )
                else:
                    nc.vector.tensor_tensor(A[:], A[:], z2eta[:], OP.add)
                    for m in range(NT):
                        nc.vector.tensor_tensor(log_s[:, m], A[:, m], CLB[:], OP.add)
                        nc.scalar.activation(Es[:, m], log_s[:, m], AF.Exp)
                if k == 0:
                    psCS2 = pp.tile([1, D], F32, tag="CS", bufs=1)
                    for kb in range(NT):
                        nc.tensor.matmul(
                            psCS2[:], lhsT=ones_k[:], rhs=Es[:, kb],
                            start=(kb == 0), stop=(kb == NT - 1),
                        )
                    nc.scalar.activation(cs_row[:], psCS2[:], AF.Copy)

                # ---- z += r*(Et - Es) ----
                dzu = bp.tile([128, NT, D], BF16, tag="Ey2")
                nc.vector.tensor_tensor(dzu[:], Et[:], Es[:], OP.subtract)
                if k == 0:
                    nc.scalar.activation(z[:], dzu[:], AF.Copy, scale=r)
                else:
                    nc.scalar.activation(dzu[:], dzu[:], AF.Copy, scale=r)
                    nc.vector.tensor_tensor(z[:], z[:], dzu[:], OP.add)

                # ---- mu block ----
                if k == 0:
                    nc.scalar.activation(rowscr[:], p0r[:], AF.Copy, accum_out=sp0[:])
                    nc.scalar.activation(l1[:], sp0[:], AF.Ln)
                    nc.vector.tensor_scalar(
                        log_mu_row[:], log_p0[:], l1[:], None, OP.subtract
                    )
                    nc.scalar.activation(rowscr[:], log_mu_row[:], AF.Exp)
                    EB = tp.tile([128, D], F32, tag="PB")
                    bcast(rowscr[:], EB[:])
                    nc.vector.tensor_scalar(rstr_neg[:], rst[:], -r, None, OP.mult)
                    for m in range(NT):
                        nc.scalar.activation(
                            z1[:, m], EB[:], AF.Identity, scale=r,
                            bias=rstr_neg[:, m : m + 1],
                        )
                elif k in (1, 2):
                    cmu = 1.0 / (r + ga)
                    if k == 1:
                        nc.vector.tensor_scalar(
                            rowscr[:], log_p0[:], ga, None, OP.mult
                        )
                        nc.vector.tensor_scalar(
                            log_mu_row[:], log_mu_row[:], r, None, OP.mult
                        )
                        nc.vector.tensor_tensor(
                            rowscr[:], rowscr[:], log_mu_row[:], OP.add
                        )
                        RB = tp.tile([128, D], F32, tag="PB")
                        bcast(rowscr[:], RB[:])
                        for m in range(NT):
                            nc.vector.tensor_tensor(
                                A[:, m], RB[:], z1[:, m], OP.subtract
                            )
                    else:
                        nc.scalar.activation(A[:], log_mu_full[:], AF.Copy, scale=r)
                        nc.vector.tensor_tensor(A[:], A[:], z1[:], OP.subtract)
                        nc.vector.tensor_scalar(
                            rowscr[:], log_p0[:], ga, None, OP.mult
                        )
                        GB = tp.tile([128, D], F32, tag="PB")
                        bcast(rowscr[:], GB[:])
                        for m in range(NT):
                            nc.vector.tensor_tensor(A[:, m], A[:, m], GB[:], OP.add)
                    Emu = bp.tile([128, NT, D], BF16, tag="Emu")
                    for m in range(NT):
                        nc.scalar.activation(
                            Emu[:, m], A[:, m], AF.Exp, scale=cmu,
                            accum_out=rs_mu[:, m : m + 1],
                        )
                    nc.scalar.activation(lse_mu[:], rs_mu[:], AF.Ln)
                    nc.vector.tensor_scalar(
                        lse_mu_neg[:], lse_mu[:], -1.0, None, OP.mult
                    )
                    for m in range(NT):
                        nc.scalar.activation(
                            log_mu_full[:, m], A[:, m], AF.Identity, scale=cmu,
                            bias=lse_mu_neg[:, m : m + 1],
                        )
                    if k == 1:
                        nc.scalar.activation(em8[:], lse_mu[:], AF.Exp, scale=-1.0)
                        nc.vector.tensor_scalar(em8r[:], em8[:], r, None, OP.mult)
                        nc.vector.tensor_scalar(
                            rstr_neg[:], rst[:], -r, None, OP.mult
                        )
                        for m in range(NT):
                            nc.scalar.activation(
                                A[:, m], Emu[:, m], AF.Identity,
                                scale=em8r[:, m : m + 1],
                                bias=rstr_neg[:, m : m + 1],
                            )
                        nc.vector.tensor_tensor(z1[:], z1[:], A[:], OP.add)

                # ---- eta block ----
                if k == 0:
                    nc.scalar.activation(col8[:], log_q0[:], AF.Exp, accum_out=pq[:])
                    ps1 = pp.tile([1, 1], F32, tag="CS", bufs=1)
                    nc.tensor.matmul(
                        ps1[:], lhsT=ones_kf[:], rhs=pq[:], start=True, stop=True
                    )
                    nc.scalar.activation(l1[:], ps1[:], AF.Ln)
                    ps2 = pp.tile([128, 1], F32, tag="BC", bufs=1)
                    nc.tensor.matmul(
                        ps2[:], lhsT=ones_m[:], rhs=l1[:], start=True, stop=True
                    )
                    nc.scalar.activation(l2c[:], ps2[:], AF.Copy)
                    nc.vector.tensor_scalar(
                        log_eta_col[:], log_q0[:], l2c[:], None, OP.subtract
                    )
                    nc.scalar.activation(ee8[:], log_eta_col[:], AF.Exp)
                    nc.vector.tensor_scalar(eer[:], ee8[:], r, None, OP.mult)
                    CSB = tp.tile([128, D], F32, tag="PB")
                    bcast(cs_row[:], CSB[:])
                    for m in range(NT):
                        nc.scalar.activation(
                            z2eta[:, m], CSB[:], AF.Identity, scale=-r,
                            bias=eer[:, m : m + 1],
                        )
                elif k == 1:
                    ceta = 1.0 / (r + de)
                    nc.vector.tensor_scalar(col8[:], log_eta_col[:], r, None, OP.mult)
                    nc.vector.tensor_scalar(ee8[:], log_q0[:], de, None, OP.mult)
                    nc.vector.tensor_tensor(col8[:], col8[:], ee8[:], OP.add)
                    nc.vector.tensor_scalar(rcolc[:], col8[:], ceta, None, OP.mult)
                    for m in range(NT):
                        nc.scalar.activation(
                            z2eta[:, m], z2eta[:, m], AF.Identity, scale=-ceta,
                            bias=rcolc[:, m : m + 1],
                        )
                    E_eta = bp.tile([128, NT, D], BF16, tag="Ey2")
                    for m in range(NT):
                        nc.scalar.activation(E_eta[:, m], z2eta[:, m], AF.Exp)
                    psCS3 = pp.tile([1, D], F32, tag="CS", bufs=1)
                    for kb in range(NT):
                        nc.tensor.matmul(
                            psCS3[:], lhsT=ones_k[:], rhs=E_eta[:, kb],
                            start=(kb == 0), stop=(kb == NT - 1),
                        )
                    nc.scalar.activation(cl_row[:], psCS3[:], AF.Ln)
                    CLB2 = tp.tile([128, D], F32, tag="PB")
                    bcast(cl_row[:], CLB2[:], scale=-1.0)
                    for m in range(NT):
                        nc.vector.tensor_tensor(
                            z2eta[:, m], z2eta[:, m], CLB2[:], OP.add
                        )
                # k == 2: eta/z1/z2 updates are dead (never read afterwards)

    return nc


def _numpy_fallback(x, c1, c2, p0, q0, a0, a1, a2, a3, rho, mask, num):
    lse_ = lambda y, ax: np.log(np.sum(np.exp(y - np.max(y, axis=ax, keepdims=True)), axis=ax, keepdims=True)) + np.max(y, axis=ax, keepdims=True)
    log_t = np.log(q0 * p0 + EPS)
    log_s = log_t.copy()
    log_mu = np.log(p0)
    log_eta = np.log(q0 + EPS)
    log_p0 = np.log(p0)
    log_q0 = np.log(q0 + EPS)
    z = np.zeros_like(log_t)
    z1 = np.zeros_like(p0)
    z2 = np.zeros_like(q0)
    for k in range(int(num)):
        n = min(k, a1.shape[0] - 1)
        tmp2 = np.matmul(np.matmul(c2, np.exp(log_s)), c1)
        y = (x + a0[n] * tmp2 - z) / rho[n] + log_s
        log_t = (log_mu - lse_(y, 2)) + y
        tmp2 = np.matmul(np.matmul(c2, np.exp(log_t)), c1)
        y = (z + a0[n] * tmp2 + rho[n] * log_t) / (a1[n] + rho[n])
        log_s = (log_eta - lse_(y, 1)) + y
        t = np.exp(log_t) * mask
        s = np.exp(log_s) * mask
        z = z + rho[n] * (t - s)
        y = (rho[n] * log_mu + a2[n] * log_p0 - z1) / (rho[n] + a2[n])
        log_mu = y - lse_(y, 2)
        y = (rho[n] * log_eta + a3[n] * log_q0 - z2) / (rho[n] + a3[n])
        log_eta = y - lse_(y, 1)
        z1 = z1 + rho[n] * (np.exp(log_mu) - np.sum(t, axis=2, keepdims=True))
        z2 = z2 + rho[n] * (np.exp(log_eta) - np.sum(s, axis=1, keepdims=True))
    return (np.exp(log_t) * mask).astype(np.float32)


def _run(nc, x, c1, c2, p0, q0, trace=False):
    from concourse.bass_utils import run_bass_kernel_spmd

    in_maps = [
        {
            "x": np.ascontiguousarray(x[b], dtype=np.float32),
            "c1": np.ascontiguousarray(c1[b], dtype=np.float32),
            "c2": np.ascontiguousarray(c2[b], dtype=np.float32),
            "p0": np.ascontiguousarray(p0[b], dtype=np.float32),
            "q0": np.ascontiguousarray(q0[b], dtype=np.float32),
        }
        for b in range(B)
    ]
    res = run_bass_kernel_spmd(nc, in_maps, core_ids=list(range(B)), trace=trace)
    out = np.stack([res.results[b]["out"] for b in range(B)]).astype(np.float32)
    return out, res


def kernel_profiled(trace=False, **inputs):
    x = np.asarray(inputs["x"], dtype=np.float32)
    c1 = np.asarray(inputs["c1"], dtype=np.float32)
    c2 = np.asarray(inputs["c2"], dtype=np.float32)
    p0 = np.asarray(inputs["p0"], dtype=np.float32)
    q0 = np.asarray(inputs["q0"], dtype=np.float32)
    a0 = np.asarray(inputs["a0"], dtype=np.float32)
    a1 = np.asarray(inputs["a1"], dtype=np.float32)
    a2 = np.asarray(inputs["a2"], dtype=np.float32)
    a3 = np.asarray(inputs["a3"], dtype=np.float32)
    rho = np.asarray(inputs["rho"], dtype=np.float32)
    mask = np.asarray(inputs["mask"], dtype=np.float32)
    num = int(np.asarray(inputs["num"]))

    if num != 4 or not np.all(mask == 1.0) or x.shape != (B, N, D):
        out = _numpy_fallback(
            x, c1, c2, p0, q0, a0, a1, a2, a3, rho, mask, num
        )
        return out, None

    params = tuple(
        (float(a0[k]), float(a1[k]), float(a2[k]), float(a3[k]), float(rho[k]))
        for k in range(4)
    )
    key = params
    if key not in _CACHE:
        _CACHE[key] = _build(params)
    nc = _CACHE[key]
    out, res = _run(nc, x, c1, c2, p0, q0, trace=trace)
    return out, res


def kernel(**inputs):
    out, _ = kernel_profiled(trace=False, **inputs)
    return out


# revision 15
# speedup vs baseline: 1.0733x; 1.0516x over previous
"""Trainium2 Bass kernel for nn_ApproxROT (entropic Bregman-ADMM OT solver).

Distribution: pure data-parallel over batch B=8 -> one batch element per
NeuronCore (8 cores). No collectives. Per core the solver runs 4 unrolled
iterations; the two big matmul chains c2 @ exp(state) @ c1 run on TensorE in
bf16, all log-domain state stays fp32 on DVE/ACT/GPSIMD.

Layout per core ("R layout"): logical (N=1024, D=512) tensors are stored as
SBUF tiles [128, 8, 512]: row i lives at partition i%128, block i//128.
Matmul operands exp(state) are transposed via a DRAM bounce + DMA-xbar
transpose (bf16) into "T layout" [128, 4, 1024] for use as lhsT.

Scalar solver parameters (a0..a3, rho) are baked into the compiled graph as
immediates at call time. logsumexp is computed without max-subtraction (the
exponents live around -13, far from fp32 overflow/underflow).
"""

import sys

sys.path.insert(0, "/opt/trn_rl_repo")

import numpy as np

N, D, B = 1024, 512, 8
NT = N // 128   # 8 row blocks  (R layout)
DT4 = D // 128  # 4 row blocks  (T layout)
EPS = 1e-8

_CACHE = {}


def _apply_waitpatch():
    # This walrus build rejects >1 sync wait command per instruction
    # ("Too many sync wait commands"). Hoist extra waits onto standalone
    # InstEventSemaphore instructions on the same engine, inserted right
    # before the instruction in its basic block.
    import concourse.mybir as mybir
    from concourse.tile import TileContext

    if getattr(TileContext, "_waitpatch_applied", False):
        return

    def split_excess_waits(nc):
        for _, bbw in list(nc.bb_map.items()):
            bb = bbw.bb if hasattr(bbw, "bb") else bbw
            out = []
            changed = False
            for inst in bb.instructions:
                si = getattr(inst, "sync_info", None)
                if si is not None and si.on_wait and len(si.on_wait) > 1:
                    waits = list(si.on_wait)
                    for w in waits[:-1]:
                        ev = mybir.InstEventSemaphore(
                            name=nc.get_next_instruction_name(), ins=[], outs=[]
                        )
                        ev.engine = inst.engine
                        ev.sync_info = mybir.SyncInfo(on_wait=[w], on_update=[])
                        nc.register_instruction(ev)
                        out.append(ev)
                    si.on_wait[:] = waits[-1:]
                    changed = True
                out.append(inst)
            if changed:
                bb.instructions = out

    _orig_exit = TileContext.__exit__

    def _patched_exit(self, exc_type, exc_val, exc_tb):
        r = _orig_exit(self, exc_type, exc_val, exc_tb)
        if exc_type is None:
            split_excess_waits(self.nc)
        return r

    TileContext.__exit__ = _patched_exit
    TileContext._waitpatch_applied = True


def _build(params):
    """params: tuple of 4 (a0, a1, a2, a3, rho) float tuples."""
    import concourse.bass as bass
    import concourse.mybir as mybir
    from concourse.tile import TileContext

    _apply_waitpatch()

    F32 = mybir.dt.float32
    BF16 = mybir.dt.bfloat16
    AF = mybir.ActivationFunctionType
    OP = mybir.AluOpType

    nc = bass.Bass()
    x_d = nc.declare_dram_parameter("x", [N, D], F32, isOutput=False)
    c1_d = nc.declare_dram_parameter("c1", [D, D], F32, isOutput=False)
    c2_d = nc.declare_dram_parameter("c2", [N, N], F32, isOutput=False)
    p0_d = nc.declare_dram_parameter("p0", [1, D], F32, isOutput=False)
    q0_d = nc.declare_dram_parameter("q0", [N, 1], F32, isOutput=False)
    out_d = nc.declare_dram_parameter("out", [N, D], F32, isOutput=True)
    e_scr = [nc.dram_tensor(f"e_scr{i}", [N, D], BF16) for i in range(2)]
    c2bf_d = nc.dram_tensor("c2bf", [N, N], BF16)

    def R(dram_ap):  # DRAM (rows, cols) -> [128, rows//128, cols] view
        return dram_ap.rearrange("(t p) j -> p t j", p=128)

    with TileContext(nc) as tc:
        with (
            tc.tile_pool(name="state", bufs=1) as sp,
            tc.tile_pool(name="bf", bufs=1) as bp,
            tc.tile_pool(name="tmp", bufs=2) as tp,
            tc.tile_pool(name="small", bufs=1) as mp,
            tc.tile_pool(name="ps", bufs=2, space="PSUM") as pp,
        ):
            # ---------------- constants / loads ----------------
            xt = sp.tile([128, NT, D], BF16, tag="x")
            nc.gpsimd.dma_start(out=xt[:], in_=R(x_d))  # casting DMA (SWDGE)

            c1t = bp.tile([128, DT4, D], BF16, tag="c1")
            for u in range(DT4):
                c1stage = tp.tile([128, D], F32, tag="c2stage")
                nc.sync.dma_start(
                    out=c1stage[:], in_=c1_d[128 * u : 128 * (u + 1), :]
                )
                nc.vector.tensor_copy(c1t[:, u], c1stage[:])

            c2T = bp.tile([128, NT, N], BF16, tag="c2T")
            for t in range(NT):
                c2stage = tp.tile([128, N], F32, tag="c2stage")
                nc.sync.dma_start(
                    out=c2stage[:], in_=c2_d[128 * t : 128 * (t + 1), :]
                )
                c2bf = tp.tile([128, N], BF16, tag="c2bfstage")
                nc.vector.tensor_copy(c2bf[:], c2stage[:])
                nc.sync.dma_start(
                    out=c2bf_d[128 * t : 128 * (t + 1), :], in_=c2bf[:]
                )
            for u in range(NT):
                nc.sync.dma_start(
                    out=c2T[:, u],
                    in_=c2bf_d[:, 128 * u : 128 * (u + 1)],
                    transpose=True,
                )

            epsc = mp.tile([128, 1], F32, tag="epsc")
            nc.vector.memset(epsc[:], EPS)
            ones_k = mp.tile([128, 1], BF16, tag="ones_k")
            nc.vector.memset(ones_k[:], 1.0)
            ones_kf = mp.tile([128, 1], F32, tag="ones_kf")
            nc.vector.memset(ones_kf[:], 1.0)
            ones_m = mp.tile([1, 128], F32, tag="ones_m")
            nc.vector.memset(ones_m[:], 1.0)

            p0r = mp.tile([1, D], F32, tag="p0r")
            nc.sync.dma_start(out=p0r[:], in_=p0_d[:])
            q0r = tp.tile([1, N], F32, tag="c2stage")
            nc.sync.dma_start(out=q0r[:], in_=q0_d.rearrange("n 1 -> 1 n"))
            q0c = mp.tile([128, NT], F32, tag="q0c")
            nc.sync.dma_start(out=q0c[:], in_=q0_d.rearrange("(t p) 1 -> p t", p=128))

            log_p0 = mp.tile([1, D], F32, tag="log_p0")
            nc.scalar.activation(log_p0[:], p0r[:], AF.Ln)
            log_q0 = mp.tile([128, NT], F32, tag="log_q0")
            nc.scalar.activation(log_q0[:], q0c[:], AF.Ln, bias=epsc[:])

            # ---------------- state ----------------
            log_s = sp.tile([128, NT, D], F32, tag="log_s")
            z = sp.tile([128, NT, D], BF16, tag="z")
            z1 = sp.tile([128, NT, D], F32, tag="z1")
            z2eta = sp.tile([128, NT, D], F32, tag="z2eta")  # z2 then log_eta_full
            log_mu_full = sp.tile([128, NT, D], F32, tag="log_mu_full")
            A = sp.tile([128, NT, D], F32, tag="A")   # y / y2 / mu / z-delta flow

            Es = bp.tile([128, NT, D], BF16, tag="Es")
            Et = bp.tile([128, NT, D], BF16, tag="Et")

            log_mu_row = mp.tile([1, D], F32, tag="log_mu_row")
            log_eta_col = mp.tile([128, NT], F32, tag="log_eta_col")
            rs = mp.tile([128, NT], F32, tag="rs")
            lse = mp.tile([128, NT], F32, tag="lse")
            rst = mp.tile([128, NT], F32, tag="rst")
            rs_mu = mp.tile([128, NT], F32, tag="rs_mu")
            lse_mu = mp.tile([128, NT], F32, tag="lse_mu")
            em8 = mp.tile([128, NT], F32, tag="em8")
            em8r = mp.tile([128, NT], F32, tag="em8r")
            eer = mp.tile([128, NT], F32, tag="eer")
            rcolc = mp.tile([128, NT], F32, tag="rcolc")
            lse_neg = mp.tile([128, NT], F32, tag="lse_neg")
            rplse_neg = mp.tile([128, NT], F32, tag="rplse_neg")
            lse_mu_neg = mp.tile([128, NT], F32, tag="lse_mu_neg")
            rstr_neg = mp.tile([128, NT], F32, tag="rstr_neg")
            eflb = mp.tile([128, NT], F32, tag="eflb")
            explse = mp.tile([128, NT], F32, tag="explse")
            ee8 = mp.tile([128, NT], F32, tag="ee8")
            col8 = mp.tile([128, NT], F32, tag="col8")
            pq = mp.tile([128, 1], F32, tag="pq")
            l2c = mp.tile([128, 1], F32, tag="l2c")
            sp0 = mp.tile([1, 1], F32, tag="sp0")
            l1 = mp.tile([1, 1], F32, tag="l1")
            rowscr = mp.tile([1, D], F32, tag="rowscr")
            cl_row = mp.tile([1, D], F32, tag="cl_row")
            cs_row = mp.tile([1, D], F32, tag="cs_row")

            def bcast(row_ap, out_sb, scale=1.0):
                ps = pp.tile([128, row_ap.shape[-1]], F32, tag="BC", bufs=1)
                nc.tensor.matmul(ps[:], lhsT=ones_m[:], rhs=row_ap, start=True, stop=True)
                nc.scalar.activation(out_sb, ps[:], AF.Copy, scale=scale)

            # ---------------- init ----------------
            for m in range(NT):
                ps = pp.tile([128, D], F32, tag="T2")
                nc.tensor.matmul(
                    ps[:], lhsT=q0r[:, 128 * m : 128 * (m + 1)], rhs=p0r[:],
                    start=True, stop=True,
                )
                nc.scalar.activation(log_s[:, m], ps[:], AF.Ln, bias=epsc[:])
                nc.scalar.activation(Es[:, m], ps[:], AF.Identity, bias=epsc[:])

            def chain(E_bf, scr, fscale, consume, fcopy_dve, fscale_ap=None):
                """tmp2' = fscale * c2 @ E @ c1; consume(m, psum_tile).
                ET[p, m, u, r] = E[128m+r, 128u+p] (xbar per-m transpose)."""
                ET = bp.tile([128, NT, DT4, 128], BF16, tag="ET")
                for m in range(NT):
                    nc.sync.dma_start(
                        out=ET[:, m], in_=E_bf[:, m], transpose=True
                    )
                Fsb = bp.tile([128, NT, D], BF16, tag="Fsb")
                for m in range(NT):
                    psF = pp.tile([128, D], F32, tag="F")
                    for u in range(DT4):
                        nc.tensor.matmul(
                            psF[:],
                            lhsT=ET[:, m, u],
                            rhs=c1t[:, u],
                            start=(u == 0),
                            stop=(u == DT4 - 1),
                        )
                    if fscale_ap is not None:
                        nc.scalar.activation(
                            Fsb[:, m], psF[:], AF.Identity,
                            scale=fscale_ap[:, m : m + 1],
                        )
                    elif fcopy_dve:
                        nc.vector.tensor_scalar(
                            Fsb[:, m], psF[:], fscale, None, OP.mult
                        )
                    else:
                        nc.scalar.activation(Fsb[:, m], psF[:], AF.Copy, scale=fscale)
                for m in range(NT):
                    psT = pp.tile([128, D], F32, tag="T2")
                    for kb in range(NT):
                        nc.tensor.matmul(
                            psT[:],
                            lhsT=c2T[:, kb, 128 * m : 128 * (m + 1)],
                            rhs=Fsb[:, kb],
                            start=(kb == 0),
                            stop=(kb == NT - 1),
                        )
                    consume(m, psT)

            dacc = mp.tile([128, 1], F32, tag="dacc")  # dummy accum for TTR

            # ---------------- iterations ----------------
            for k in range(4):
                al, be, ga, de, r = params[k]
                last = k == 3
                split_prev = k in (1, 2)  # log_s state is w + eta_col bias

                # ---- v = (x - z)/r + log_s  (into A) ----
                if k == 0:
                    nc.vector.tensor_scalar(A[:], xt[:], 1.0 / r, None, OP.mult)
                else:
                    dz = bp.tile([128, NT, D], BF16, tag="Ey2")
                    nc.vector.tensor_tensor(dz[:], xt[:], z[:], OP.subtract)
                    nc.scalar.activation(A[:], dz[:], AF.Copy, scale=1.0 / r)
                nc.vector.tensor_tensor(A[:], A[:], log_s[:], OP.add)
                if split_prev:
                    eta_bias = log_q0 if k == 1 else log_eta_col
                    for m in range(NT):
                        nc.scalar.activation(
                            A[:, m], A[:, m], AF.Identity,
                            bias=eta_bias[:, m : m + 1],
                        )

                # ---- chain A; y = v + tmp2'; rowsum exp; A := y + mu;
                #      E't = exp(y+mu) (lse folded later into F-scale) ----
                if k <= 1:
                    mu_row = log_p0 if k == 0 else log_mu_row
                    PB = tp.tile([128, D], F32, tag="PB")
                    bcast(mu_row[:], PB[:])

                def consumeA(m, psT):
                    nc.vector.tensor_tensor(A[:, m], A[:, m], psT[:], OP.add)
                    scr = tp.tile([128, D], BF16, tag="escr")
                    nc.scalar.activation(
                        scr[:], A[:, m], AF.Exp, accum_out=rs[:, m : m + 1]
                    )
                    if k <= 1:
                        nc.vector.tensor_tensor(A[:, m], A[:, m], PB[:], OP.add)
                    else:
                        nc.vector.tensor_tensor(
                            A[:, m], A[:, m], log_mu_full[:, m], OP.add
                        )
                    if not last:
                        nc.scalar.activation(
                            Et[:, m], A[:, m], AF.Exp,
                            accum_out=(rst[:, m : m + 1] if k <= 1 else None),
                        )

                chain(Es, e_scr[0], al / r, consumeA, fcopy_dve=False)
                nc.scalar.activation(lse[:], rs[:], AF.Ln)
                nc.vector.tensor_scalar(lse_neg[:], lse[:], -1.0, None, OP.mult)

                if last:
                    # output = exp(log_t) (mask all-ones); into log_s buffer
                    for m in range(NT):
                        nc.scalar.activation(
                            log_s[:, m], A[:, m], AF.Exp,
                            bias=lse_neg[:, m : m + 1],
                        )
                    nc.sync.dma_start(out=R(out_d), in_=log_s[:])
                    break

                nc.scalar.activation(explse[:], lse[:], AF.Exp, scale=-1.0)
                # rs currently holds sum(exp(y+mu)) per row? no: rs = sum(exp(y));
                # row sums of t = exp(-lse) * sum_j exp(y+mu): accumulate below.

                # ---- log_s := rp*(A - lse)  (= rp*log_t, the y2 seed) ----
                cb = 1.0 / (be + r)
                rp = r / (be + r)
                nc.vector.tensor_scalar(
                    rplse_neg[:], lse[:], -rp, None, OP.mult
                )
                for m in range(NT):
                    nc.scalar.activation(
                        log_s[:, m], A[:, m], AF.Identity, scale=rp,
                        bias=rplse_neg[:, m : m + 1],
                    )

                # ---- q2 into A ----
                if k == 0:
                    nc.vector.tensor_copy(A[:], log_s[:])
                else:
                    nc.scalar.activation(A[:], z[:], AF.Copy, scale=cb)
                    nc.vector.tensor_tensor(A[:], A[:], log_s[:], OP.add)

                # ---- chain B; y2 = q2 + tmp2'; Ey2 = exp(y2) ----
                Ey2 = bp.tile([128, NT, D], BF16, tag="Ey2")

                def consumeB(m, psT):
                    nc.vector.tensor_tensor(A[:, m], A[:, m], psT[:], OP.add)
                    nc.scalar.activation(Ey2[:, m], A[:, m], AF.Exp)

                nc.vector.tensor_scalar(
                    eflb[:], explse[:], al / (be + r), None, OP.mult
                )
                chain(Et, e_scr[1], 0.0, consumeB, fcopy_dve=False,
                      fscale_ap=eflb)

                # ---- col lse ----
                psCS = pp.tile([1, D], F32, tag="CS", bufs=1)
                for kb in range(NT):
                    nc.tensor.matmul(
                        psCS[:], lhsT=ones_k[:], rhs=Ey2[:, kb],
                        start=(kb == 0), stop=(kb == NT - 1),
                    )
                nc.scalar.activation(cl_row[:], psCS[:], AF.Ln)
                CLB = tp.tile([128, D], F32, tag="PB")
                bcast(cl_row[:], CLB[:], scale=-1.0)

                # ---- log_s(-w) = y2 - clse ; Es = exp(log_s [+ eta bias]) ----
                if k <= 1:
                    eta_bias = log_q0 if k == 0 else log_eta_col
                    for m in range(NT):
                        nc.vector.tensor_tensor(log_s[:, m], A[:, m], CLB[:], OP.add)
                        nc.scalar.activation(
                            Es[:, m], log_s[:, m], AF.Exp,
                            bias=eta_bias[:, m : m + 1],
                        )
                else:
                    nc.vector.tensor_tensor(A[:], A[:], z2eta[:], OP.add)
                    for m in range(NT):
                        nc.vector.tensor_tensor(log_s[:, m], A[:, m], CLB[:], OP.add)
                        nc.scalar.activation(Es[:, m], log_s[:, m], AF.Exp)
                if k == 0:
                    psCS2 = pp.tile([1, D], F32, tag="CS", bufs=1)
                    for kb in range(NT):
                        nc.tensor.matmul(
                            psCS2[:], lhsT=ones_k[:], rhs=Es[:, kb],
                            start=(kb == 0), stop=(kb == NT - 1),
                        )
                    nc.scalar.activation(cs_row[:], psCS2[:], AF.Copy)

                # ---- z += r*(Et - Es) ----
                for m in range(NT):
                    nc.scalar.activation(
                        Et[:, m], Et[:, m], AF.Identity,
                        scale=explse[:, m : m + 1],
                    )
                dzu = bp.tile([128, NT, D], BF16, tag="Ey2")
                nc.vector.tensor_tensor(dzu[:], Et[:], Es[:], OP.subtract)
                if k == 0:
                    nc.scalar.activation(z[:], dzu[:], AF.Copy, scale=r)
                else:
                    nc.scalar.activation(dzu[:], dzu[:], AF.Copy, scale=r)
                    nc.vector.tensor_tensor(z[:], z[:], dzu[:], OP.add)

                # ---- mu block ----
                if k == 0:
                    nc.scalar.activation(rowscr[:], p0r[:], AF.Copy, accum_out=sp0[:])
                    nc.scalar.activation(l1[:], sp0[:], AF.Ln)
                    nc.vector.tensor_scalar(
                        log_mu_row[:], log_p0[:], l1[:], None, OP.subtract
                    )
                    nc.scalar.activation(rowscr[:], log_mu_row[:], AF.Exp)
                    EB = tp.tile([128, D], F32, tag="PB")
                    bcast(rowscr[:], EB[:])
                    nc.vector.tensor_tensor(rstr_neg[:], rst[:], explse[:], OP.mult)
                    nc.vector.tensor_scalar(rstr_neg[:], rstr_neg[:], -r, None, OP.mult)
                    for m in range(NT):
                        nc.scalar.activation(
                            z1[:, m], EB[:], AF.Identity, scale=r,
                            bias=rstr_neg[:, m : m + 1],
                        )
                elif k in (1, 2):
                    cmu = 1.0 / (r + ga)
                    if k == 1:
                        nc.vector.tensor_scalar(
                            rowscr[:], log_p0[:], ga, None, OP.mult
                        )
                        nc.vector.tensor_scalar(
                            log_mu_row[:], log_mu_row[:], r, None, OP.mult
                        )
                        nc.vector.tensor_tensor(
                            rowscr[:], rowscr[:], log_mu_row[:], OP.add
                        )
                        RB = tp.tile([128, D], F32, tag="PB")
                        bcast(rowscr[:], RB[:])
                        for m in range(NT):
                            nc.vector.tensor_tensor(
                                A[:, m], RB[:], z1[:, m], OP.subtract
                            )
                    else:
                        nc.scalar.activation(A[:], log_mu_full[:], AF.Copy, scale=r)
                        nc.vector.tensor_tensor(A[:], A[:], z1[:], OP.subtract)
                        nc.vector.tensor_scalar(
                            rowscr[:], log_p0[:], ga, None, OP.mult
                        )
                        GB = tp.tile([128, D], F32, tag="PB")
                        bcast(rowscr[:], GB[:])
                        for m in range(NT):
                            nc.vector.tensor_tensor(A[:, m], A[:, m], GB[:], OP.add)
                    Emu = bp.tile([128, NT, D], BF16, tag="Emu")
                    for m in range(NT):
                        nc.scalar.activation(
                            Emu[:, m], A[:, m], AF.Exp, scale=cmu,
                            accum_out=rs_mu[:, m : m + 1],
                        )
                    nc.scalar.activation(lse_mu[:], rs_mu[:], AF.Ln)
                    nc.vector.tensor_scalar(
                        lse_mu_neg[:], lse_mu[:], -1.0, None, OP.mult
                    )
                    for m in range(NT):
                        nc.scalar.activation(
                            log_mu_full[:, m], A[:, m], AF.Identity, scale=cmu,
                            bias=lse_mu_neg[:, m : m + 1],
                        )
                    if k == 1:
                        nc.scalar.activation(em8[:], lse_mu[:], AF.Exp, scale=-1.0)
                        nc.vector.tensor_scalar(em8r[:], em8[:], r, None, OP.mult)
                        nc.vector.tensor_tensor(
                            rstr_neg[:], rst[:], explse[:], OP.mult
                        )
                        nc.vector.tensor_scalar(
                            rstr_neg[:], rstr_neg[:], -r, None, OP.mult
                        )
                        for m in range(NT):
                            nc.scalar.activation(
                                A[:, m], Emu[:, m], AF.Identity,
                                scale=em8r[:, m : m + 1],
                                bias=rstr_neg[:, m : m + 1],
                            )
                        nc.vector.tensor_tensor(z1[:], z1[:], A[:], OP.add)

                # ---- eta block ----
                if k == 0:
                    nc.scalar.activation(col8[:], log_q0[:], AF.Exp, accum_out=pq[:])
                    ps1 = pp.tile([1, 1], F32, tag="CS", bufs=1)
                    nc.tensor.matmul(
                        ps1[:], lhsT=ones_kf[:], rhs=pq[:], start=True, stop=True
                    )
                    nc.scalar.activation(l1[:], ps1[:], AF.Ln)
                    ps2 = pp.tile([128, 1], F32, tag="BC", bufs=1)
                    nc.tensor.matmul(
                        ps2[:], lhsT=ones_m[:], rhs=l1[:], start=True, stop=True
                    )
                    nc.scalar.activation(l2c[:], ps2[:], AF.Copy)
                    nc.vector.tensor_scalar(
                        log_eta_col[:], log_q0[:], l2c[:], None, OP.subtract
                    )
                    nc.scalar.activation(ee8[:], log_eta_col[:], AF.Exp)
                    nc.vector.tensor_scalar(eer[:], ee8[:], r, None, OP.mult)
                    CSB = tp.tile([128, D], F32, tag="PB")
                    bcast(cs_row[:], CSB[:])
                    for m in range(NT):
                        nc.scalar.activation(
                            z2eta[:, m], CSB[:], AF.Identity, scale=-r,
                            bias=eer[:, m : m + 1],
                        )
                elif k == 1:
                    ceta = 1.0 / (r + de)
                    nc.vector.tensor_scalar(col8[:], log_eta_col[:], r, None, OP.mult)
                    nc.vector.tensor_scalar(ee8[:], log_q0[:], de, None, OP.mult)
                    nc.vector.tensor_tensor(col8[:], col8[:], ee8[:], OP.add)
                    nc.vector.tensor_scalar(rcolc[:], col8[:], ceta, None, OP.mult)
                    for m in range(NT):
                        nc.scalar.activation(
                            z2eta[:, m], z2eta[:, m], AF.Identity, scale=-ceta,
                            bias=rcolc[:, m : m + 1],
                        )
                    E_eta = bp.tile([128, NT, D], BF16, tag="Ey2")
                    for m in range(NT):
                        nc.scalar.activation(E_eta[:, m], z2eta[:, m], AF.Exp)
                    psCS3 = pp.tile([1, D], F32, tag="CS", bufs=1)
                    for kb in range(NT):
                        nc.tensor.matmul(
                            psCS3[:], lhsT=ones_k[:], rhs=E_eta[:, kb],
                            start=(kb == 0), stop=(kb == NT - 1),
                        )
                    nc.scalar.activation(cl_row[:], psCS3[:], AF.Ln)
                    CLB2 = tp.tile([128, D], F32, tag="PB")
                    bcast(cl_row[:], CLB2[:], scale=-1.0)
                    for m in range(NT):
                        nc.vector.tensor_tensor(
                            z2eta[:, m], z2eta[:, m], CLB2[:], OP.add
                        )
                # k == 2: eta/z1/z2 updates are dead (never read afterwards)

    return nc


def _numpy_fallback(x, c1, c2, p0, q0, a0, a1, a2, a3, rho, mask, num):
    lse_ = lambda y, ax: np.log(np.sum(np.exp(y - np.max(y, axis=ax, keepdims=True)), axis=ax, keepdims=True)) + np.max(y, axis=ax, keepdims=True)
    log_t = np.log(q0 * p0 + EPS)
    log_s = log_t.copy()
    log_mu = np.log(p0)
    log_eta = np.log(q0 + EPS)
    log_p0 = np.log(p0)
    log_q0 = np.log(q0 + EPS)
    z = np.zeros_like(log_t)
    z1 = np.zeros_like(p0)
    z2 = np.zeros_like(q0)
    for k in range(int(num)):
        n = min(k, a1.shape[0] - 1)
        tmp2 = np.matmul(np.matmul(c2, np.exp(log_s)), c1)
        y = (x + a0[n] * tmp2 - z) / rho[n] + log_s
        log_t = (log_mu - lse_(y, 2)) + y
        tmp2 = np.matmul(np.matmul(c2, np.exp(log_t)), c1)
        y = (z + a0[n] * tmp2 + rho[n] * log_t) / (a1[n] + rho[n])
        log_s = (log_eta - lse_(y, 1)) + y
        t = np.exp(log_t) * mask
        s = np.exp(log_s) * mask
        z = z + rho[n] * (t - s)
        y = (rho[n] * log_mu + a2[n] * log_p0 - z1) / (rho[n] + a2[n])
        log_mu = y - lse_(y, 2)
        y = (rho[n] * log_eta + a3[n] * log_q0 - z2) / (rho[n] + a3[n])
        log_eta = y - lse_(y, 1)
        z1 = z1 + rho[n] * (np.exp(log_mu) - np.sum(t, axis=2, keepdims=True))
        z2 = z2 + rho[n] * (np.exp(log_eta) - np.sum(s, axis=1, keepdims=True))
    return (np.exp(log_t) * mask).astype(np.float32)


def _run(nc, x, c1, c2, p0, q0, trace=False):
    from concourse.bass_utils import run_bass_kernel_spmd

    in_maps = [
        {
            "x": np.ascontiguousarray(x[b], dtype=np.float32),
            "c1": np.ascontiguousarray(c1[b], dtype=np.float32),
            "c2": np.ascontiguousarray(c2[b], dtype=np.float32),
            "p0": np.ascontiguousarray(p0[b], dtype=np.float32),
            "q0": np.ascontiguousarray(q0[b], dtype=np.float32),
        }
        for b in range(B)
    ]
    res = run_bass_kernel_spmd(nc, in_maps, core_ids=list(range(B)), trace=trace)
    out = np.stack([res.results[b]["out"] for b in range(B)]).astype(np.float32)
    return out, res


def kernel_profiled(trace=False, **inputs):
    x = np.asarray(inputs["x"], dtype=np.float32)
    c1 = np.asarray(inputs["c1"], dtype=np.float32)
    c2 = np.asarray(inputs["c2"], dtype=np.float32)
    p0 = np.asarray(inputs["p0"], dtype=np.float32)
    q0 = np.asarray(inputs["q0"], dtype=np.float32)
    a0 = np.asarray(inputs["a0"], dtype=np.float32)
    a1 = np.asarray(inputs["a1"], dtype=np.float32)
    a2 = np.asarray(inputs["a2"], dtype=np.float32)
    a3 = np.asarray(inputs["a3"], dtype=np.float32)
    rho = np.asarray(inputs["rho"], dtype=np.float32)
    mask = np.asarray(inputs["mask"], dtype=np.float32)
    num = int(np.asarray(inputs["num"]))

    if num != 4 or not np.all(mask == 1.0) or x.shape != (B, N, D):
        out = _numpy_fallback(
            x, c1, c2, p0, q0, a0, a1, a2, a3, rho, mask, num
        )
        return out, None

    params = tuple(
        (float(a0[k]), float(a1[k]), float(a2[k]), float(a3[k]), float(rho[k]))
        for k in range(4)
    )
    key = params
    if key not in _CACHE:
        _CACHE[key] = _build(params)
    nc = _CACHE[key]
    out, res = _run(nc, x, c1, c2, p0, q0, trace=trace)
    return out, res


def kernel(**inputs):
    out, _ = kernel_profiled(trace=False, **inputs)
    return out
